# revision 12
# baseline (speedup 1.0000x reference)
"""Multi-head attention (B=4, S=2048, D=2048, H=16) on 8 trn2 NeuronCores.

Sharding: 4 head-groups x 2 batch-groups. Core c handles heads
[(c//2)*4, (c//2)*4+4) for batches [(c%2)*2, (c%2)*2+2). Each core computes
its heads' Q/K/V projections, full causal+padding-masked attention, and a
partial output projection; the host sums the 4 partial outputs per batch.

All matmuls run in float32r (reduced-precision fp32, ~bf16 PE rate when
pipelined, ~1.5e-4 rel err). Attention uses a transposed-scores layout
st[k, q] so the key-padding mask folds into the exp() bias (per-partition)
and exp tiles feed the attn@V matmul directly as the moving operand. The
softmax denominator comes from an all-ones stationary matmul that leaves the
row sums replicated across all 128 PSUM partitions, so the division happens
with plain elementwise DVE ops. Rows whose causally-visible keys are all
masked reproduce the reference's uniform-attention fallback via a
host-precomputed mean-of-V fixup.

Inputs are host-retiled into contiguous [128, 512]-class blocks so every DMA
is a few large descriptors instead of 128 row-fragments.
"""

import os
import sys

import numpy as np

sys.path.insert(0, "/opt/trn_rl_repo")

B, S, D, H, DK = 4, 2048, 2048, 16, 128
NHG = 4  # head groups (cores along head axis)
NBG = 2  # batch groups
HPC = H // NHG  # heads per core = 4
BPC = B // NBG  # batches per core = 2
NI = D // 128  # contraction blocks = 16
NSC = S // 512  # 512-wide s-chunks = 4
NST = S // 128  # 128-wide s-tiles = 16
SCALE = 1.0 / float(np.sqrt(DK))
NEGB = -30000.0

_cache = {}


def _register_ntff_hook():
    """The agent image lacks antenv.axon_hooks; register the NTFF profile
    hook manually so trace=True can report HW exec time."""
    import types

    if "antenv.axon_hooks" in sys.modules:
        return
    try:
        import trn_agent_boot.trn_boot as _tb

        hook = _tb._ntff_profile_via_ctypes("/opt/axon/libaxon_pjrt.so")
    except Exception:
        hook = None
    m = types.ModuleType("antenv.axon_hooks")
    m.get_axon_ntff_profile_hook = lambda: hook
    m.set_axon_ntff_profile_hook = lambda h: None
    sys.modules["antenv.axon_hooks"] = m


def _split_waits(nc):
    """This container's walrus accepts a single sync-wait per instruction.
    Hoist extra waits onto EventSemaphore instructions placed immediately
    before the over-subscribed instruction on the same engine."""
    import concourse.mybir as mb

    ctr = 0
    for f in nc.m.functions:
        for blk in f.blocks:
            new = []
            for inst in blk.instructions:
                si = inst.sync_info
                waits = list(si.on_wait) if (si and si.on_wait) else []
                if len(waits) > 1:
                    for w in waits[:-1]:
                        ctr += 1
                        ev = mb.InstEventSemaphore(
                            name=f"WSPLIT-{ctr}", ins=[], outs=[]
                        )
                        ev.engine = inst.engine
                        ev.sync_info = mb.SyncInfo(on_wait=[w], on_update=[])
                        new.append(ev)
                    si.on_wait = [waits[-1]]
                new.append(inst)
            blk.instructions[:] = new
    return ctr


def _build_program():
    import concourse.bass as bass
    import concourse.mybir as mybir
    import concourse.tile as tile
    from contextlib import ExitStack

    f32 = mybir.dt.float32
    f32r = mybir.dt.float32r
    bf16 = mybir.dt.bfloat16
    EXP = mybir.ActivationFunctionType.Exp
    ADD = mybir.AluOpType.add
    MUL = mybir.AluOpType.mult
    ISEQ = mybir.AluOpType.is_equal

    nc = bass.Bass()
    # host-retiled contiguous blocks
    xtd = nc.dram_tensor(
        "xtt", [BPC, NI, NSC, 128, 512], f32r, kind="ExternalInput"
    ).ap()
    xbd = nc.dram_tensor(
        "xtb", [BPC, NI, NSC, 128, 512], bf16, kind="ExternalInput"
    ).ap()
    wqkd = nc.dram_tensor(
        "wqkt", [NI, 128, 2 * HPC * DK], bf16, kind="ExternalInput"
    ).ap()
    wvd = nc.dram_tensor(
        "wvt", [NI, 128, HPC * DK], f32r, kind="ExternalInput"
    ).ap()
    wod = nc.dram_tensor(
        "wott", [NSC, HPC, 128, 512], f32r, kind="ExternalInput"
    ).ap()
    mbd = nc.dram_tensor("mb", [BPC, 128, NST], f32, kind="ExternalInput").ap()
    patd = nc.dram_tensor("pat", [4, 128, 512], f32r, kind="ExternalInput").ap()
    idend = nc.dram_tensor("iden", [128, 128], f32r, kind="ExternalInput").ap()
    mvd = nc.dram_tensor("meanv", [BPC, 128, HPC], f32, kind="ExternalInput").ap()
    onesd = nc.dram_tensor("ones", [128, 128], f32r, kind="ExternalInput").ap()
    outd = nc.dram_tensor("out", [BPC, S, D], f32, kind="ExternalOutput").ap()

    with tile.TileContext(nc) as tc, ExitStack() as ctx:
        singles = ctx.enter_context(tc.tile_pool(name="singles", bufs=1))
        pers = ctx.enter_context(tc.tile_pool(name="pers", bufs=1))
        xs = ctx.enter_context(tc.tile_pool(name="xs", bufs=5))
        ws = ctx.enter_context(tc.tile_pool(name="ws", bufs=4))
        expp = ctx.enter_context(tc.tile_pool(name="expp", bufs=4))
        smp = ctx.enter_context(tc.tile_pool(name="smp", bufs=2))
        outp = ctx.enter_context(tc.tile_pool(name="outp", bufs=2))

        pat_sb = singles.tile([128, 4, 512], f32r)
        iden_sb = singles.tile([128, 128], f32r)
        nc.sync.dma_start(out=iden_sb, in_=idend)
        for p in range(4):
            nc.sync.dma_start(out=pat_sb[:, p, :], in_=patd[p])
        mb_sb = singles.tile([128, BPC, NST], f32)
        mv_sb = singles.tile([128, BPC, HPC], f32)
        for b in range(BPC):
            nc.sync.dma_start(out=mb_sb[:, b, :], in_=mbd[b])
            nc.sync.dma_start(out=mv_sb[:, b, :], in_=mvd[b])
        ones_sb = singles.tile([128, 128], f32r)
        nc.sync.dma_start(out=ones_sb, in_=onesd)

        for b in range(BPC):
            # persistent per-batch tiles (slots reused across batches)
            qt = [pers.tile([128, S], bf16, name=f"qt{h}", bufs=2) for h in range(HPC)]
            kt = [pers.tile([128, S], bf16, name=f"kt{h}", bufs=2) for h in range(HPC)]
            v_sb = pers.tile([128, NST, HPC * DK], f32r, name="v_sb")
            ot = [pers.tile([128, S], f32r, name=f"ot{h}") for h in range(HPC)]

            # ---- Q/K projections: qt/kt[h] = (x[b] @ w.T).T slices ----
            with nc.named_scope(f"projQK_b{b}"):
                with tc.tile_pool(name="psA", bufs=1, space="PSUM") as psA:
                    for sc in range(NSC):
                        pq = [
                            psA.tile([128, 512], f32, name=f"pq{h}")
                            for h in range(HPC)
                        ]
                        pk = [
                            psA.tile([128, 512], f32, name=f"pk{h}")
                            for h in range(HPC)
                        ]
                        for ib in range(NI):
                            xt_blk = xs.tile([128, 512], bf16, name="xb_blk")
                            nc.sync.dma_start(out=xt_blk, in_=xbd[b, ib, sc])
                            wqk_blk = ws.tile(
                                [128, 2 * HPC * DK], bf16, name="wqk_blk"
                            )
                            nc.sync.dma_start(out=wqk_blk, in_=wqkd[ib])
                            for h in range(HPC):
                                nc.tensor.matmul(
                                    pq[h][:, :],
                                    wqk_blk[:, h * DK : (h + 1) * DK],
                                    xt_blk[:, :],
                                    start=(ib == 0),
                                    stop=(ib == NI - 1),
                                )
                                nc.tensor.matmul(
                                    pk[h][:, :],
                                    wqk_blk[
                                        :,
                                        HPC * DK + h * DK : HPC * DK + (h + 1) * DK,
                                    ],
                                    xt_blk[:, :],
                                    start=(ib == 0),
                                    stop=(ib == NI - 1),
                                )
                        for h in range(HPC):
                            nc.scalar.copy(
                                qt[h][:, sc * 512 : (sc + 1) * 512], pq[h][:, :]
                            )
                            nc.vector.tensor_copy(
                                kt[h][:, sc * 512 : (sc + 1) * 512], pk[h][:, :]
                            )

            # ---- V projection (natural layout [s, dv]) ----
            with nc.named_scope(f"projV_b{b}"):
                with tc.tile_pool(name="psB", bufs=1, space="PSUM") as psB:
                    for stg2 in range(2):
                        pv = [
                            psB.tile([128, HPC * DK], f32, name=f"pv{j}")
                            for j in range(8)
                        ]
                        for ib in range(NI):
                            wv_blk = ws.tile([128, HPC * DK], f32r, name="wv_blk")
                            nc.sync.dma_start(out=wv_blk, in_=wvd[ib])
                            for g in range(2):
                                xv_blk = xs.tile([128, 512], f32r, name="xt_blk")
                                nc.sync.dma_start(
                                    out=xv_blk, in_=xtd[b, ib, stg2 * 2 + g]
                                )
                                for j in range(4):
                                    nc.tensor.matmul(
                                        pv[g * 4 + j][:, :],
                                        xv_blk[:, j * 128 : (j + 1) * 128],
                                        wv_blk[:, :],
                                        start=(ib == 0),
                                        stop=(ib == NI - 1),
                                    )
                        for j in range(8):
                            nc.scalar.copy(
                                v_sb[:, stg2 * 8 + j, :], pv[j][:, :]
                            )

            # ---- attention per head ----
            with nc.named_scope(f"attn_b{b}"):
                with tc.tile_pool(name="psS", bufs=3, space="PSUM") as psS, \
                     tc.tile_pool(name="psO", bufs=2, space="PSUM") as psO, \
                     tc.tile_pool(name="psD", bufs=2, space="PSUM") as psD:
                    for h in range(HPC):
                        pd_sb = smp.tile([128, S], f32, name="pd_sb", bufs=1)
                        for qc in range(NSC):
                            nkb = (qc + 1) * 4
                            po = psO.tile([128, 512], f32, name="po")
                            pd = psD.tile([128, 512], f32, name="pd")
                            for kb in range(nkb):
                                ps = psS.tile([128, 512], f32, name="ps")
                                p = kb - qc * 4
                                if p >= 0:
                                    nc.tensor.matmul(
                                        ps[:, :],
                                        iden_sb[:, :],
                                        pat_sb[:, p, :],
                                        start=True,
                                        stop=False,
                                    )
                                nc.tensor.matmul(
                                    ps[:, :],
                                    kt[h][:, kb * 128 : (kb + 1) * 128],
                                    qt[h][:, qc * 512 : (qc + 1) * 512],
                                    start=(p < 0),
                                    stop=True,
                                )
                                e = expp.tile([128, 512], f32r, name="e")
                                nc.scalar.activation(
                                    out=e[:, :],
                                    in_=ps[:, :],
                                    func=EXP,
                                    bias=mb_sb[:, b, kb : kb + 1],
                                    scale=SCALE,
                                )
                                nc.tensor.matmul(
                                    po[:, :],
                                    v_sb[:, kb, h * DK : (h + 1) * DK],
                                    e[:, :],
                                    start=(kb == 0),
                                    stop=(kb == nkb - 1),
                                )
                                nc.tensor.matmul(
                                    pd[:, :],
                                    ones_sb[:, :],
                                    e[:, :],
                                    start=(kb == 0),
                                    stop=(kb == nkb - 1),
                                )
                            # drain PSUM quickly; normalize later in bulk
                            nc.vector.tensor_copy(
                                ot[h][:, qc * 512 : (qc + 1) * 512], po[:, :]
                            )
                            nc.vector.tensor_copy(
                                pd_sb[:, qc * 512 : (qc + 1) * 512], pd[:, :]
                            )
                        # bulk normalize ot[h] (off the PE critical chain)
                        for hf in range(2):
                            sl = slice(hf * 1024, (hf + 1) * 1024)
                            zm = smp.tile([128, 1024], f32, name="zm")
                            nc.vector.tensor_scalar(
                                out=zm[:, :],
                                in0=pd_sb[:, sl],
                                scalar1=0.0,
                                scalar2=None,
                                op0=ISEQ,
                            )
                            nc.vector.tensor_tensor(
                                pd_sb[:, sl], pd_sb[:, sl], zm[:, :], ADD
                            )
                            nc.vector.reciprocal(pd_sb[:, sl], pd_sb[:, sl])
                            nc.vector.tensor_tensor(
                                ot[h][:, sl], ot[h][:, sl], pd_sb[:, sl], MUL
                            )
                            nc.vector.scalar_tensor_tensor(
                                out=ot[h][:, sl],
                                in0=zm[:, :],
                                scalar=mv_sb[:, b, h : h + 1],
                                in1=ot[h][:, sl],
                                op0=MUL,
                                op1=ADD,
                            )

            # ---- output projection (partial over this core's heads) ----
            with nc.named_scope(f"projO_b{b}"):
                with tc.tile_pool(name="psF", bufs=2, space="PSUM") as psF, \
                     tc.tile_pool(name="wop", bufs=1) as wop:
                    for ec in range(NSC):
                        wot_blk = wop.tile([128, HPC, 512], f32r, name="wot_blk")
                        for hd in range(HPC):
                            nc.sync.dma_start(
                                out=wot_blk[:, hd, :], in_=wod[ec, hd]
                            )
                        for st in range(NST):
                            pf = psF.tile([128, 512], f32, name="pf")
                            for h in range(HPC):
                                nc.tensor.matmul(
                                    pf[:, :],
                                    ot[h][:, st * 128 : (st + 1) * 128],
                                    wot_blk[:, h, :],
                                    start=(h == 0),
                                    stop=(h == HPC - 1),
                                )
                            ob = outp.tile([128, 512], f32, name="ob")
                            nc.vector.tensor_copy(ob[:, :], pf[:, :])
                            nc.scalar.dma_start(
                                out=outd[
                                    b,
                                    st * 128 : (st + 1) * 128,
                                    ec * 512 : (ec + 1) * 512,
                                ],
                                in_=ob[:, :],
                            )

    _split_waits(nc)
    return nc


def _host_prep(x, attention_mask, w_q, w_k, w_v, w_o):
    x = np.asarray(x, dtype=np.float32)
    mask = np.asarray(attention_mask)
    w_q = np.asarray(w_q, dtype=np.float32)
    w_k = np.asarray(w_k, dtype=np.float32)
    w_v = np.asarray(w_v, dtype=np.float32)
    w_o = np.asarray(w_o, dtype=np.float32)

    import ml_dtypes

    xt = x.transpose(0, 2, 1)  # [B, D, S] view
    # [B, NI, 128, NSC, 512] -> [B, NI, NSC, 128, 512]
    xtt = np.ascontiguousarray(
        xt.reshape(B, NI, 128, NSC, 512).transpose(0, 1, 3, 2, 4)
    )
    xtb = xtt.astype(ml_dtypes.bfloat16)

    wqT = w_q.T  # [i, d_out] view
    wkT = w_k.T
    wvT = w_v.T
    woT = w_o.T  # [hd, e] view

    m01 = mask.astype(np.float32)  # [B, S]
    mb = (NEGB * (1.0 - m01)).reshape(B, NST, 128).transpose(0, 2, 1)
    mb = np.ascontiguousarray(mb)  # [B, 128, NST]

    ki = np.arange(128)[:, None]
    qj = np.arange(512)[None, :]
    pat = np.zeros((4, 128, 512), dtype=np.float32)
    for p in range(4):
        pat[p] = np.where(p * 128 + ki <= qj, 0.0, NEGB)

    # mean of V rows over ALL keys, for the all-masked-row fallback
    xsum = x.sum(axis=1)  # [B, D]
    mv_full = (xsum @ w_v.T) / float(S)  # [B, D]

    ones = np.ones((128, 128), dtype=np.float32)
    iden = np.eye(128, dtype=np.float32)

    in_maps = []
    xtt_slices = [
        np.ascontiguousarray(xtt[bg * BPC : (bg + 1) * BPC]) for bg in range(NBG)
    ]
    xtb_slices = [
        np.ascontiguousarray(xtb[bg * BPC : (bg + 1) * BPC]) for bg in range(NBG)
    ]
    mb_slices = [
        np.ascontiguousarray(mb[bg * BPC : (bg + 1) * BPC]) for bg in range(NBG)
    ]
    for c in range(8):
        hg, bg = c // 2, c % 2
        cols = slice(hg * HPC * DK, (hg + 1) * HPC * DK)
        # [NI, 128, 1024]: wq cols then wk cols per i-block
        wqk = np.concatenate([wqT[:, cols], wkT[:, cols]], axis=1)
        wqkt = np.ascontiguousarray(
            wqk.reshape(NI, 128, 2 * HPC * DK)
        ).astype(ml_dtypes.bfloat16)
        wvt = np.ascontiguousarray(wvT[:, cols].reshape(NI, 128, HPC * DK))
        # wott[ec, hd] = woT[this core's hd rows, ec-block] as [128, 512]
        wo_rows = woT[cols, :]  # [512, 2048]
        wott = np.ascontiguousarray(
            wo_rows.reshape(HPC, 128, NSC, 512).transpose(2, 0, 1, 3)
        )
        mv = mv_full[bg * BPC : (bg + 1) * BPC, cols]  # [BPC, 512]
        mv = np.ascontiguousarray(
            mv.reshape(BPC, HPC, DK).transpose(0, 2, 1)
        )  # [BPC, 128, HPC]
        in_maps.append(
            {
                "xtt": xtt_slices[bg],
                "xtb": xtb_slices[bg],
                "wqkt": wqkt,
                "wvt": wvt,
                "wott": wott,
                "mb": mb_slices[bg],
                "pat": pat,
                "meanv": mv,
                "ones": ones,
                "iden": iden,
            }
        )
    return in_maps


def kernel(x, attention_mask, w_q, w_k, w_v, w_o):
    _register_ntff_hook()
    from concourse.bass_utils import run_bass_kernel_spmd

    if "nc" not in _cache:
        _cache["nc"] = _build_program()
    nc = _cache["nc"]

    in_maps = _host_prep(x, attention_mask, w_q, w_k, w_v, w_o)

    trace = bool(int(os.environ.get("BASS_KERNEL_TRACE", "0")))
    res = run_bass_kernel_spmd(
        nc, in_maps, core_ids=list(range(8)), trace=trace
    )
    _cache["last_exec_time_ns"] = res.exec_time_ns
    _cache["last_results"] = res

    out = np.zeros((B, S, D), dtype=np.float32)
    for c in range(8):
        hg, bg = c // 2, c % 2
        part = res.results[c]["out"]  # [BPC, S, D]
        out[bg * BPC : (bg + 1) * BPC] += part
    return out


# revision 13
# speedup vs baseline: 1.0096x; 1.0096x over previous
"""Multi-head attention (B=4, S=2048, D=2048, H=16) on 8 trn2 NeuronCores.

Sharding: 4 head-groups x 2 batch-groups. Core c handles heads
[(c//2)*4, (c//2)*4+4) for batches [(c%2)*2, (c%2)*2+2). Each core computes
its heads' Q/K/V projections, full causal+padding-masked attention, and a
partial output projection; the host sums the 4 partial outputs per batch.

All matmuls run in float32r (reduced-precision fp32, ~bf16 PE rate when
pipelined, ~1.5e-4 rel err). Attention uses a transposed-scores layout
st[k, q] so the key-padding mask folds into the exp() bias (per-partition)
and exp tiles feed the attn@V matmul directly as the moving operand. The
softmax denominator comes from an all-ones stationary matmul that leaves the
row sums replicated across all 128 PSUM partitions, so the division happens
with plain elementwise DVE ops. Rows whose causally-visible keys are all
masked reproduce the reference's uniform-attention fallback via a
host-precomputed mean-of-V fixup.

Inputs are host-retiled into contiguous [128, 512]-class blocks so every DMA
is a few large descriptors instead of 128 row-fragments.
"""

import os
import sys

import numpy as np

sys.path.insert(0, "/opt/trn_rl_repo")

B, S, D, H, DK = 4, 2048, 2048, 16, 128
NHG = 4  # head groups (cores along head axis)
NBG = 2  # batch groups
HPC = H // NHG  # heads per core = 4
BPC = B // NBG  # batches per core = 2
NI = D // 128  # contraction blocks = 16
NSC = S // 512  # 512-wide s-chunks = 4
NST = S // 128  # 128-wide s-tiles = 16
SCALE = 1.0 / float(np.sqrt(DK))
NEGB = -30000.0

_cache = {}


def _register_ntff_hook():
    """The agent image lacks antenv.axon_hooks; register the NTFF profile
    hook manually so trace=True can report HW exec time."""
    import types

    if "antenv.axon_hooks" in sys.modules:
        return
    try:
        import trn_agent_boot.trn_boot as _tb

        hook = _tb._ntff_profile_via_ctypes("/opt/axon/libaxon_pjrt.so")
    except Exception:
        hook = None
    m = types.ModuleType("antenv.axon_hooks")
    m.get_axon_ntff_profile_hook = lambda: hook
    m.set_axon_ntff_profile_hook = lambda h: None
    sys.modules["antenv.axon_hooks"] = m


def _split_waits(nc):
    """This container's walrus accepts a single sync-wait per instruction.
    Hoist extra waits onto EventSemaphore instructions placed immediately
    before the over-subscribed instruction on the same engine."""
    import concourse.mybir as mb

    ctr = 0
    for f in nc.m.functions:
        for blk in f.blocks:
            new = []
            for inst in blk.instructions:
                si = inst.sync_info
                waits = list(si.on_wait) if (si and si.on_wait) else []
                if len(waits) > 1:
                    for w in waits[:-1]:
                        ctr += 1
                        ev = mb.InstEventSemaphore(
                            name=f"WSPLIT-{ctr}", ins=[], outs=[]
                        )
                        ev.engine = inst.engine
                        ev.sync_info = mb.SyncInfo(on_wait=[w], on_update=[])
                        new.append(ev)
                    si.on_wait = [waits[-1]]
                new.append(inst)
            blk.instructions[:] = new
    return ctr


def _build_program():
    import concourse.bass as bass
    import concourse.mybir as mybir
    import concourse.tile as tile
    from contextlib import ExitStack

    f32 = mybir.dt.float32
    f32r = mybir.dt.float32r
    bf16 = mybir.dt.bfloat16
    EXP = mybir.ActivationFunctionType.Exp
    ADD = mybir.AluOpType.add
    MUL = mybir.AluOpType.mult
    ISEQ = mybir.AluOpType.is_equal

    nc = bass.Bass()
    # host-retiled contiguous blocks
    xtd = nc.dram_tensor(
        "xtt", [BPC, NI, NSC, 128, 512], f32r, kind="ExternalInput"
    ).ap()
    xbd = nc.dram_tensor(
        "xtb", [BPC, NI, NSC, 128, 512], bf16, kind="ExternalInput"
    ).ap()
    wqkd = nc.dram_tensor(
        "wqkt", [NI, 128, 2 * HPC * DK], bf16, kind="ExternalInput"
    ).ap()
    wvd = nc.dram_tensor(
        "wvt", [NI, 128, HPC * DK], f32r, kind="ExternalInput"
    ).ap()
    wod = nc.dram_tensor(
        "wott", [NSC, HPC, 128, 512], f32r, kind="ExternalInput"
    ).ap()
    mbd = nc.dram_tensor("mb", [BPC, 128, NST], f32, kind="ExternalInput").ap()
    patd = nc.dram_tensor("pat", [4, 128, 512], f32r, kind="ExternalInput").ap()
    idend = nc.dram_tensor("iden", [128, 128], f32r, kind="ExternalInput").ap()
    mvd = nc.dram_tensor("meanv", [BPC, 128, HPC], f32, kind="ExternalInput").ap()
    onesd = nc.dram_tensor("ones", [128, 128], f32r, kind="ExternalInput").ap()
    outd = nc.dram_tensor("out", [BPC, S, D], f32, kind="ExternalOutput").ap()

    with tile.TileContext(nc) as tc, ExitStack() as ctx:
        singles = ctx.enter_context(tc.tile_pool(name="singles", bufs=1))
        pers = ctx.enter_context(tc.tile_pool(name="pers", bufs=1))
        xs = ctx.enter_context(tc.tile_pool(name="xs", bufs=6))
        ws = ctx.enter_context(tc.tile_pool(name="ws", bufs=4))
        expp = ctx.enter_context(tc.tile_pool(name="expp", bufs=4))
        smp = ctx.enter_context(tc.tile_pool(name="smp", bufs=2))
        outp = ctx.enter_context(tc.tile_pool(name="outp", bufs=2))

        pat_sb = singles.tile([128, 4, 512], f32r)
        iden_sb = singles.tile([128, 128], f32r)
        nc.sync.dma_start(out=iden_sb, in_=idend)
        for p in range(4):
            nc.sync.dma_start(out=pat_sb[:, p, :], in_=patd[p])
        mb_sb = singles.tile([128, BPC, NST], f32)
        mv_sb = singles.tile([128, BPC, HPC], f32)
        for b in range(BPC):
            nc.sync.dma_start(out=mb_sb[:, b, :], in_=mbd[b])
            nc.sync.dma_start(out=mv_sb[:, b, :], in_=mvd[b])
        ones_sb = singles.tile([128, 128], f32r)
        nc.sync.dma_start(out=ones_sb, in_=onesd)

        for b in range(BPC):
            # persistent per-batch tiles (slots reused across batches)
            qt = [pers.tile([128, S], bf16, name=f"qt{h}") for h in range(HPC)]
            kt = [pers.tile([128, S], bf16, name=f"kt{h}") for h in range(HPC)]
            v_sb = pers.tile([128, NST, HPC * DK], f32r, name="v_sb")
            ot = [pers.tile([128, S], f32r, name=f"ot{h}") for h in range(HPC)]

            # ---- Q/K projections: qt/kt[h] = (x[b] @ w.T).T slices ----
            with nc.named_scope(f"projQK_b{b}"):
                with tc.tile_pool(name="psA", bufs=1, space="PSUM") as psA:
                    wqk_sb = pers.tile(
                        [128, NI, 2 * HPC * DK], bf16, name="wqk_sb"
                    )
                    for sc in range(NSC):
                        pq = [
                            psA.tile([128, 512], f32, name=f"pq{h}")
                            for h in range(HPC)
                        ]
                        pk = [
                            psA.tile([128, 512], f32, name=f"pk{h}")
                            for h in range(HPC)
                        ]
                        for ib in range(NI):
                            xt_blk = xs.tile([128, 512], bf16, name="xb_blk")
                            nc.sync.dma_start(out=xt_blk, in_=xbd[b, ib, sc])
                            if sc == 0:
                                nc.sync.dma_start(
                                    out=wqk_sb[:, ib, :], in_=wqkd[ib]
                                )
                            wqk_blk = wqk_sb[:, ib, :]
                            for h in range(HPC):
                                nc.tensor.matmul(
                                    pq[h][:, :],
                                    wqk_blk[:, h * DK : (h + 1) * DK],
                                    xt_blk[:, :],
                                    start=(ib == 0),
                                    stop=(ib == NI - 1),
                                )
                                nc.tensor.matmul(
                                    pk[h][:, :],
                                    wqk_blk[
                                        :,
                                        HPC * DK + h * DK : HPC * DK + (h + 1) * DK,
                                    ],
                                    xt_blk[:, :],
                                    start=(ib == 0),
                                    stop=(ib == NI - 1),
                                )
                        for h in range(HPC):
                            nc.scalar.copy(
                                qt[h][:, sc * 512 : (sc + 1) * 512], pq[h][:, :]
                            )
                            nc.vector.tensor_copy(
                                kt[h][:, sc * 512 : (sc + 1) * 512], pk[h][:, :]
                            )

            # ---- V projection (natural layout [s, dv]) ----
            with nc.named_scope(f"projV_b{b}"):
                with tc.tile_pool(name="psB", bufs=1, space="PSUM") as psB:
                    for stg2 in range(2):
                        pv = [
                            psB.tile([128, HPC * DK], f32, name=f"pv{j}")
                            for j in range(8)
                        ]
                        for ib in range(NI):
                            wv_blk = ws.tile([128, HPC * DK], f32r, name="wv_blk")
                            nc.sync.dma_start(out=wv_blk, in_=wvd[ib])
                            for g in range(2):
                                xv_blk = xs.tile([128, 512], f32r, name="xt_blk")
                                nc.sync.dma_start(
                                    out=xv_blk, in_=xtd[b, ib, stg2 * 2 + g]
                                )
                                for j in range(4):
                                    nc.tensor.matmul(
                                        pv[g * 4 + j][:, :],
                                        xv_blk[:, j * 128 : (j + 1) * 128],
                                        wv_blk[:, :],
                                        start=(ib == 0),
                                        stop=(ib == NI - 1),
                                    )
                        for j in range(8):
                            nc.scalar.copy(
                                v_sb[:, stg2 * 8 + j, :], pv[j][:, :]
                            )

            # ---- attention per head ----
            with nc.named_scope(f"attn_b{b}"):
                with tc.tile_pool(name="psS", bufs=3, space="PSUM") as psS, \
                     tc.tile_pool(name="psO", bufs=2, space="PSUM") as psO, \
                     tc.tile_pool(name="psD", bufs=2, space="PSUM") as psD:
                    for h in range(HPC):
                        pd_sb = smp.tile([128, S], f32, name="pd_sb")
                        for qc in range(NSC):
                            nkb = (qc + 1) * 4
                            po = psO.tile([128, 512], f32, name="po")
                            pd = psD.tile([128, 512], f32, name="pd")
                            for kb in range(nkb):
                                ps = psS.tile([128, 512], f32, name="ps")
                                p = kb - qc * 4
                                if p >= 0:
                                    nc.tensor.matmul(
                                        ps[:, :],
                                        iden_sb[:, :],
                                        pat_sb[:, p, :],
                                        start=True,
                                        stop=False,
                                    )
                                nc.tensor.matmul(
                                    ps[:, :],
                                    kt[h][:, kb * 128 : (kb + 1) * 128],
                                    qt[h][:, qc * 512 : (qc + 1) * 512],
                                    start=(p < 0),
                                    stop=True,
                                )
                                e = expp.tile([128, 512], f32r, name="e")
                                nc.scalar.activation(
                                    out=e[:, :],
                                    in_=ps[:, :],
                                    func=EXP,
                                    bias=mb_sb[:, b, kb : kb + 1],
                                    scale=SCALE,
                                )
                                nc.tensor.matmul(
                                    po[:, :],
                                    v_sb[:, kb, h * DK : (h + 1) * DK],
                                    e[:, :],
                                    start=(kb == 0),
                                    stop=(kb == nkb - 1),
                                )
                                nc.tensor.matmul(
                                    pd[:, :],
                                    ones_sb[:, :],
                                    e[:, :],
                                    start=(kb == 0),
                                    stop=(kb == nkb - 1),
                                )
                            # drain PSUM quickly; normalize later in bulk
                            nc.vector.tensor_copy(
                                ot[h][:, qc * 512 : (qc + 1) * 512], po[:, :]
                            )
                            nc.vector.tensor_copy(
                                pd_sb[:, qc * 512 : (qc + 1) * 512], pd[:, :]
                            )
                        # bulk normalize ot[h] (off the PE critical chain)
                        for hf in range(2):
                            sl = slice(hf * 1024, (hf + 1) * 1024)
                            zm = smp.tile([128, 1024], f32, name="zm")
                            nc.vector.tensor_scalar(
                                out=zm[:, :],
                                in0=pd_sb[:, sl],
                                scalar1=0.0,
                                scalar2=None,
                                op0=ISEQ,
                            )
                            nc.vector.tensor_tensor(
                                pd_sb[:, sl], pd_sb[:, sl], zm[:, :], ADD
                            )
                            nc.vector.reciprocal(pd_sb[:, sl], pd_sb[:, sl])
                            nc.vector.tensor_tensor(
                                ot[h][:, sl], ot[h][:, sl], pd_sb[:, sl], MUL
                            )
                            nc.vector.scalar_tensor_tensor(
                                out=ot[h][:, sl],
                                in0=zm[:, :],
                                scalar=mv_sb[:, b, h : h + 1],
                                in1=ot[h][:, sl],
                                op0=MUL,
                                op1=ADD,
                            )

            # ---- output projection (partial over this core's heads) ----
            with nc.named_scope(f"projO_b{b}"):
                with tc.tile_pool(name="psF", bufs=2, space="PSUM") as psF, \
                     tc.tile_pool(name="wop", bufs=1) as wop:
                    for ec in range(NSC):
                        wot_blk = wop.tile([128, HPC, 512], f32r, name="wot_blk")
                        for hd in range(HPC):
                            nc.sync.dma_start(
                                out=wot_blk[:, hd, :], in_=wod[ec, hd]
                            )
                        for st in range(NST):
                            pf = psF.tile([128, 512], f32, name="pf")
                            for h in range(HPC):
                                nc.tensor.matmul(
                                    pf[:, :],
                                    ot[h][:, st * 128 : (st + 1) * 128],
                                    wot_blk[:, h, :],
                                    start=(h == 0),
                                    stop=(h == HPC - 1),
                                )
                            ob = outp.tile([128, 512], f32, name="ob")
                            nc.vector.tensor_copy(ob[:, :], pf[:, :])
                            nc.scalar.dma_start(
                                out=outd[
                                    b,
                                    st * 128 : (st + 1) * 128,
                                    ec * 512 : (ec + 1) * 512,
                                ],
                                in_=ob[:, :],
                            )

    _split_waits(nc)
    return nc


def _host_prep(x, attention_mask, w_q, w_k, w_v, w_o):
    x = np.asarray(x, dtype=np.float32)
    mask = np.asarray(attention_mask)
    w_q = np.asarray(w_q, dtype=np.float32)
    w_k = np.asarray(w_k, dtype=np.float32)
    w_v = np.asarray(w_v, dtype=np.float32)
    w_o = np.asarray(w_o, dtype=np.float32)

    import ml_dtypes

    xt = x.transpose(0, 2, 1)  # [B, D, S] view
    # [B, NI, 128, NSC, 512] -> [B, NI, NSC, 128, 512]
    xtt = np.ascontiguousarray(
        xt.reshape(B, NI, 128, NSC, 512).transpose(0, 1, 3, 2, 4)
    )
    xtb = xtt.astype(ml_dtypes.bfloat16)

    wqT = w_q.T  # [i, d_out] view
    wkT = w_k.T
    wvT = w_v.T
    woT = w_o.T  # [hd, e] view

    m01 = mask.astype(np.float32)  # [B, S]
    mb = (NEGB * (1.0 - m01)).reshape(B, NST, 128).transpose(0, 2, 1)
    mb = np.ascontiguousarray(mb)  # [B, 128, NST]

    ki = np.arange(128)[:, None]
    qj = np.arange(512)[None, :]
    pat = np.zeros((4, 128, 512), dtype=np.float32)
    for p in range(4):
        pat[p] = np.where(p * 128 + ki <= qj, 0.0, NEGB)

    # mean of V rows over ALL keys, for the all-masked-row fallback
    xsum = x.sum(axis=1)  # [B, D]
    mv_full = (xsum @ w_v.T) / float(S)  # [B, D]

    ones = np.ones((128, 128), dtype=np.float32)
    iden = np.eye(128, dtype=np.float32)

    in_maps = []
    xtt_slices = [
        np.ascontiguousarray(xtt[bg * BPC : (bg + 1) * BPC]) for bg in range(NBG)
    ]
    xtb_slices = [
        np.ascontiguousarray(xtb[bg * BPC : (bg + 1) * BPC]) for bg in range(NBG)
    ]
    mb_slices = [
        np.ascontiguousarray(mb[bg * BPC : (bg + 1) * BPC]) for bg in range(NBG)
    ]
    for c in range(8):
        hg, bg = c // 2, c % 2
        cols = slice(hg * HPC * DK, (hg + 1) * HPC * DK)
        # [NI, 128, 1024]: wq cols then wk cols per i-block
        wqk = np.concatenate([wqT[:, cols], wkT[:, cols]], axis=1)
        wqkt = np.ascontiguousarray(
            wqk.reshape(NI, 128, 2 * HPC * DK)
        ).astype(ml_dtypes.bfloat16)
        wvt = np.ascontiguousarray(wvT[:, cols].reshape(NI, 128, HPC * DK))
        # wott[ec, hd] = woT[this core's hd rows, ec-block] as [128, 512]
        wo_rows = woT[cols, :]  # [512, 2048]
        wott = np.ascontiguousarray(
            wo_rows.reshape(HPC, 128, NSC, 512).transpose(2, 0, 1, 3)
        )
        mv = mv_full[bg * BPC : (bg + 1) * BPC, cols]  # [BPC, 512]
        mv = np.ascontiguousarray(
            mv.reshape(BPC, HPC, DK).transpose(0, 2, 1)
        )  # [BPC, 128, HPC]
        in_maps.append(
            {
                "xtt": xtt_slices[bg],
                "xtb": xtb_slices[bg],
                "wqkt": wqkt,
                "wvt": wvt,
                "wott": wott,
                "mb": mb_slices[bg],
                "pat": pat,
                "meanv": mv,
                "ones": ones,
                "iden": iden,
            }
        )
    return in_maps


def kernel(x, attention_mask, w_q, w_k, w_v, w_o):
    _register_ntff_hook()
    from concourse.bass_utils import run_bass_kernel_spmd

    if "nc" not in _cache:
        _cache["nc"] = _build_program()
    nc = _cache["nc"]

    in_maps = _host_prep(x, attention_mask, w_q, w_k, w_v, w_o)

    trace = bool(int(os.environ.get("BASS_KERNEL_TRACE", "0")))
    res = run_bass_kernel_spmd(
        nc, in_maps, core_ids=list(range(8)), trace=trace
    )
    _cache["last_exec_time_ns"] = res.exec_time_ns
    _cache["last_results"] = res

    out = np.zeros((B, S, D), dtype=np.float32)
    for c in range(8):
        hg, bg = c // 2, c % 2
        part = res.results[c]["out"]  # [BPC, S, D]
        out[bg * BPC : (bg + 1) * BPC] += part
    return out
